# revision 26
# baseline (speedup 1.0000x reference)
"""Trainium2 Bass kernel for nn_AggrSum (segment_sum of H rows by X_node).

out[v, :] = sum_{n : X_node[n] == v} H[n, :],  H [1600000, 128] f32,
X_node [1600000] int64 in [0, 100000).

Strategy (8 NeuronCores, SPMD single program):
  * Host planning: argsort X_node; the V axis is tiled into 64-segment
    sub-windows, paired antithetically by row count into PSUM-tile
    pairs (A at partitions 0-63, B at 64-127) and dealt round-robin by
    pair size to (core, slot).  Rows of a pair pack DENSELY (A rows
    then B rows, padding only at pair granularity -> ~3.5% padding
    instead of ~7%).  A fixed "zone" of chunks around the per-core A/B
    crossover gets BOTH one-hot compares, keeping the program SPMD.
    H streams as fp16 (2 B/elem, ~5e-4 error vs the 2e-2 gate).
  * Device: gather groups of ~GCH chunks stream via one split DMA on
    the two HWDGE rings; per group TWO DVE is_equal ops build fp8
    one-hot blocks (A range vs iota 0..63, B range vs iota 64..127).
    One-hot tiles get one buffer per range (all stay resident),
    decoupling DVE from buffer recycling.  Per chunk one matmul
    (lhsT=onehot [128, 64] fp8, rhs=H chunk [128, 128] fp16)
    accumulates the pair's [64, 128] f32 PSUM half (PE tile_position
    0/64).  ACT copies PSUM -> fp16 staging; batched output DMAs
    drain on the otherwise idle GPSIMD SWDGE ring.
  * Host scatters the per-core pair blocks back to V order.

Segment-sharded output means no cross-core reduction; each core
streams ~1/8 of the rows once (~53 MB) and writes 3.2 MB.
"""
import dataclasses

import numpy as np

import concourse.bass as bass
import concourse.mybir as mybir
import concourse.tile as tile
from concourse import bacc
from concourse import bass_utils

P = 128
D = 128
PSEG = 64
OUT_BATCH = 8             # PSUM tiles per output staging DMA
GCH = 48                  # target chunks per gather DMA (~1.6 MB)
N_CORES = 8
F32 = mybir.dt.float32
F16 = mybir.dt.float16
F8 = mybir.dt.float8e4

_CACHE = {}


def _plan_schedule(X, n_cores):
    N = X.shape[0]
    V = int(X.max()) + 1 if N else 1
    perm = np.argsort(X)
    Xs = X[perm].astype(np.int64)

    nws = -(-V // PSEG)
    win_of_node = Xs // PSEG
    counts = np.bincount(win_of_node, minlength=nws)[:nws]
    starts = np.zeros(nws + 1, dtype=np.int64)
    np.cumsum(counts, out=starts[1:])

    # antithetic pairing: largest with smallest -> near-uniform pair sums
    srt = np.argsort(-counts, kind="stable")
    npairs_raw = -(-nws // 2)
    PW = -(-npairs_raw // n_cores)
    npairs = PW * n_cores
    pairA = np.full(npairs, -1, dtype=np.int64)
    pairB = np.full(npairs, -1, dtype=np.int64)
    pairA[:npairs_raw] = srt[:npairs_raw]
    nbw = nws - npairs_raw
    if nbw > 0:
        pairB[:nbw] = srt[::-1][:nbw]
    pcnt = np.where(pairA >= 0, counts[np.clip(pairA, 0, None)], 0) + \
        np.where(pairB >= 0, counts[np.clip(pairB, 0, None)], 0)

    psrt = np.argsort(-pcnt, kind="stable")
    assign = psrt.reshape(PW, n_cores)   # pair index per (slot, core)

    # per (slot, core): cntA, cnt_pair
    aw = pairA[assign]
    bw = pairB[assign]
    cA = np.where(aw >= 0, counts[np.clip(aw, 0, None)], 0)
    cB = np.where(bw >= 0, counts[np.clip(bw, 0, None)], 0)
    cP = cA + cB
    KA = (cA.min(axis=1) // P).astype(np.int64)             # pure-A chunks
    KB = (-(-cA.max(axis=1) // P)).astype(np.int64)         # A ends by here
    KP = np.maximum.reduce([
        np.ones(PW, dtype=np.int64),
        KB,
        (-(-cP.max(axis=1) // P)).astype(np.int64),
    ])
    KA = np.minimum(KA, KP)
    KB = np.minimum(np.maximum(KB, KA), KP)

    # gather groups of pair-slots, <= cap stream chunks; first two small
    off = np.zeros(PW + 1, dtype=np.int64)
    np.cumsum(KP, out=off[1:])
    groups = []
    s0 = 0
    while s0 < PW:
        rem = int(off[PW] - off[s0])
        if len(groups) < 2 or rem <= 2 * GCH:
            cap = 16
        else:
            cap = GCH
        s1 = s0
        while s1 < PW and off[s1 + 1] - off[s0] <= cap:
            s1 += 1
        if s1 == s0:
            s1 = s0 + 1
        groups.append((s0, s1))
        s0 = s1

    # stream layout: per group [pure-A blocks][zone blocks][pure-B blocks]
    TOT = int(off[PW])
    chunk_pos = np.zeros((PW, int(KP.max())), dtype=np.int64)
    ginfo = []
    t = 0
    for (s0, s1) in groups:
        t0 = t
        nA = int(sum(KA[s] for s in range(s0, s1)))
        nz = int(sum(KB[s] - KA[s] for s in range(s0, s1)))
        nB_ = int(sum(KP[s] - KB[s] for s in range(s0, s1)))
        pa, pz, pb = t0, t0 + nA, t0 + nA + nz
        for s in range(s0, s1):
            for j in range(int(KP[s])):
                if j < KA[s]:
                    chunk_pos[s, j] = pa
                    pa += 1
                elif j < KB[s]:
                    chunk_pos[s, j] = pz
                    pz += 1
                else:
                    chunk_pos[s, j] = pb
                    pb += 1
        t = t0 + nA + nz + nB_
        ginfo.append(dict(t0=t0, nch=nA + nz + nB_, lenA=nA + nz,
                          zrel=nA, lenB=nz + nB_, s0=s0, s1=s1))
    assert t == TOT

    order = np.full((n_cores, TOT * P), -1, dtype=np.int64)
    xrel = np.full((n_cores, P, TOT), -1.0, dtype=np.float32)
    for c in range(n_cores):
        ov = order[c].reshape(TOT, P)
        xr = xrel[c]
        for s in range(PW):
            rows = []
            vals = []
            for wi, base_off in ((int(aw[s, c]), 0), (int(bw[s, c]), PSEG)):
                if wi < 0:
                    continue
                a, b = int(starts[wi]), int(starts[wi + 1])
                rows.append(perm[a:b])
                vals.append((Xs[a:b] - wi * PSEG + base_off).astype(
                    np.float32))
            kp = int(KP[s])
            rr = np.full(kp * P, -1, dtype=np.int64)
            vv = np.full(kp * P, -1.0, dtype=np.float32)
            if rows:
                rcat = np.concatenate(rows)
                vcat = np.concatenate(vals)
                rr[:len(rcat)] = rcat
                vv[:len(vcat)] = vcat
            rr = rr.reshape(kp, P)
            vv = vv.reshape(kp, P)
            for j in range(kp):
                pos = int(chunk_pos[s, j])
                ov[pos] = rr[j]
                xr[:, pos] = vv[j]

    iota = np.ascontiguousarray(np.broadcast_to(
        np.arange(2 * PSEG, dtype=np.float16)[None, :], (P, 2 * PSEG)))

    return dict(
        V=V, PW=PW, KA=KA, KB=KB, KP=KP, TOT=TOT, n_cores=n_cores,
        groups=groups, ginfo=ginfo, chunk_pos=chunk_pos,
        aw=aw, bw=bw, order=order,
        xrel=xrel.astype(np.float16), iota=iota,
    )


def _make_in_maps(H, meta):
    n_cores, TOT = meta["n_cores"], meta["TOT"]
    H16 = H.astype(np.float16)
    maps = []
    for c in range(n_cores):
        flat = meta["order"][c]
        sel = np.clip(flat, 0, None)
        hh = H16[sel]
        hh[flat < 0] = 0
        hh = np.ascontiguousarray(hh.reshape(TOT, P, D).transpose(1, 0, 2))
        maps.append({
            "h": hh,
            "xrel": meta["xrel"][c],
            "iota": meta["iota"],
        })
    return maps


def _assemble_output(res_outs, meta):
    n_cores, PW, V = meta["n_cores"], meta["PW"], meta["V"]
    aw, bw = meta["aw"], meta["bw"]
    full = np.zeros((-(-V // PSEG) * PSEG + PSEG, D), dtype=np.float32)
    for c in range(n_cores):
        # out layout: [P, PW*D] fp16; partitions 0-63 = A, 64-127 = B
        oc = res_outs[c].astype(np.float32).reshape(P, PW, D)
        for s in range(PW):
            wa, wb = int(aw[s, c]), int(bw[s, c])
            if wa >= 0:
                full[wa * PSEG:(wa + 1) * PSEG] = oc[:PSEG, s]
            if wb >= 0:
                full[wb * PSEG:(wb + 1) * PSEG] = oc[PSEG:, s]
    return full[:V]


def _ap3(ap, mid, inner):
    # replace the free dims of a 2d AP with [mid, inner] ([step, count])
    part = ap.ap[0]
    new = [part, list(mid), list(inner)]
    return dataclasses.replace(ap, ap=new)


def _build_nc(meta, nbufs=6):
    PW, TOT = meta["PW"], meta["TOT"]
    KA = [int(k) for k in meta["KA"]]
    KB = [int(k) for k in meta["KB"]]
    KP = [int(k) for k in meta["KP"]]
    cpos = meta["chunk_pos"]
    n_cores = meta["n_cores"]
    nc = bacc.Bacc("TRN2", target_bir_lowering=False, debug=False,
                   num_devices=n_cores)
    h = nc.dram_tensor("h", [P, TOT, D], F16, kind="ExternalInput").ap()
    xrel_d = nc.dram_tensor("xrel", [P, TOT], F16, kind="ExternalInput").ap()
    iota_d = nc.dram_tensor("iota", [P, 2 * PSEG], F16,
                            kind="ExternalInput").ap()
    out_d = nc.dram_tensor("out", [P, PW * D], F16,
                           kind="ExternalOutput").ap()

    with tile.TileContext(nc) as tc:
        with (
            tc.tile_pool(name="res", bufs=1) as res,
            tc.tile_pool(name="gat", bufs=nbufs) as gat,
            tc.tile_pool(name="oh", bufs=min(24, 2 * len(meta["groups"]))) as ohp,
            tc.tile_pool(name="ps", bufs=4, space="PSUM") as ps,
            tc.tile_pool(name="osb", bufs=3) as osb,
        ):
            xrel_sb = res.tile([P, TOT], F16)
            iota_sb = res.tile([P, 2 * PSEG], F16)
            nc.gpsimd.dma_start(out=xrel_sb[:], in_=xrel_d[:])
            nc.gpsimd.dma_start(out=iota_sb[:], in_=iota_d[:])

            staging = None
            for gi, g in enumerate(meta["ginfo"]):
                t0, nch = g["t0"], g["nch"]
                gt = gat.tile([P, nch * D], F16, tag="gt")
                h1 = nch // 2
                nc.sync.dma_start(
                    out=gt[:, :h1 * D],
                    in_=h[:, t0:t0 + h1, :].rearrange("p t d -> p (t d)"))
                nc.scalar.dma_start(
                    out=gt[:, h1 * D:],
                    in_=h[:, t0 + h1:t0 + nch, :].rearrange(
                        "p t d -> p (t d)"))
                lenA, zrel, lenB = g["lenA"], g["zrel"], g["lenB"]
                ohA = ohp.tile([P, max(lenA, 1) * PSEG], F8, tag="ohA")
                ohB = ohp.tile([P, max(lenB, 1) * PSEG], F8, tag="ohB")
                if lenA:
                    nc.vector.tensor_tensor(
                        out=ohA[:, :lenA * PSEG],
                        in0=_ap3(iota_sb[:, :PSEG], [0, lenA], [1, PSEG]),
                        in1=_ap3(xrel_sb[:, t0:t0 + lenA],
                                 [1, lenA], [0, PSEG]),
                        op=mybir.AluOpType.is_equal,
                    )
                if lenB:
                    nc.vector.tensor_tensor(
                        out=ohB[:, :lenB * PSEG],
                        in0=_ap3(iota_sb[:, PSEG:], [0, lenB], [1, PSEG]),
                        in1=_ap3(xrel_sb[:, t0 + zrel:t0 + zrel + lenB],
                                 [1, lenB], [0, PSEG]),
                        op=mybir.AluOpType.is_equal,
                    )
                for s in range(g["s0"], g["s1"]):
                    pt = ps.tile([P, D], F32, tag="pt")
                    # fallback chunks keep the slice inside the oh tile that
                    # covers them (empty side -> all-pad or ignored output)
                    if KB[s] > 0:
                        a_side = (range(KB[s]), ohA, t0, 0)
                    else:
                        a_side = ([KP[s] - 1], ohB, t0 + zrel, 0)
                    if KP[s] > KA[s]:
                        b_side = (range(KA[s], KP[s]), ohB, t0 + zrel, PSEG)
                    else:
                        b_side = ([KP[s] - 1], ohA, t0, PSEG)
                    for chunks, oh, rng0, poff in (a_side, b_side):
                        for ji, j in enumerate(chunks):
                            pos = int(cpos[s, j])
                            rel = pos - rng0
                            nc.tensor.matmul(
                                out=pt[poff:poff + PSEG, :],
                                lhsT=oh[:, rel * PSEG:(rel + 1) * PSEG],
                                rhs=gt[:, (pos - t0) * D:(pos - t0 + 1) * D],
                                start=(ji == 0),
                                stop=(ji == len(chunks) - 1),
                                tile_position=(0, poff),
                            )
                    b = s % OUT_BATCH
                    if b == 0:
                        staging = osb.tile([P, OUT_BATCH * D], F16, tag="st")
                    nc.scalar.copy(out=staging[:, b * D:(b + 1) * D],
                                   in_=pt[:])
                    if b == OUT_BATCH - 1 or s == PW - 1:
                        g0 = s - b
                        nc.gpsimd.dma_start(
                            out=out_d[:, g0 * D:(s + 1) * D],
                            in_=staging[:, :(b + 1) * D])

    nc.compile()
    return nc


def prepare(H, X_node):
    """Plan + build + shard. Returns (nc, in_maps, meta). Cached on the
    schedule signature so repeated kernel() calls reuse the compiled
    program."""
    H = np.ascontiguousarray(np.asarray(H, dtype=np.float32))
    X = np.asarray(X_node).astype(np.int64)
    assert H.ndim == 2 and H.shape[1] == D and X.shape == (H.shape[0],)

    meta = _plan_schedule(X, N_CORES)
    key = (meta["PW"], meta["TOT"],
           tuple(int(k) for k in meta["KA"]),
           tuple(int(k) for k in meta["KB"]),
           tuple(int(k) for k in meta["KP"]))
    if key not in _CACHE:
        _CACHE[key] = _build_nc(meta)
    nc = _CACHE[key]
    in_maps = _make_in_maps(H, meta)
    return nc, in_maps, meta


def kernel(H, X_node):
    nc, in_maps, meta = prepare(H, X_node)
    res = bass_utils.run_bass_kernel_spmd(
        nc, in_maps, core_ids=list(range(N_CORES)))
    out = _assemble_output([res.results[c]["out"] for c in range(N_CORES)],
                           meta)
    return out.astype(np.float32)


# revision 27
# speedup vs baseline: 1.0979x; 1.0979x over previous
"""Trainium2 Bass kernel for nn_AggrSum (segment_sum of H rows by X_node).

out[v, :] = sum_{n : X_node[n] == v} H[n, :],  H [1600000, 128] f32,
X_node [1600000] int64 in [0, 100000).

Strategy (8 NeuronCores, SPMD single program):
  * Host planning: argsort X_node; the V axis is tiled into 64-segment
    sub-windows, paired antithetically by row count into PSUM-tile
    pairs (A at partitions 0-63, B at 64-127) and dealt round-robin by
    pair size to (core, slot).  Rows of a pair pack DENSELY (A rows
    then B rows, padding only at pair granularity -> ~3.5% padding
    instead of ~7%).  A fixed "zone" of chunks around the per-core A/B
    crossover gets BOTH one-hot compares, keeping the program SPMD.
    H streams as fp16 (2 B/elem, ~5e-4 error vs the 2e-2 gate).
  * Device: gather groups of ~GCH chunks stream via one split DMA on
    the two HWDGE rings; per group TWO DVE is_equal ops build fp8
    one-hot blocks (A range vs iota 0..63, B range vs iota 64..127).
    One-hot tiles get one buffer per range (all stay resident),
    decoupling DVE from buffer recycling.  Per chunk one matmul
    (lhsT=onehot [128, 64] fp8, rhs=H chunk [128, 128] fp16)
    accumulates the pair's [64, 128] f32 PSUM half (PE tile_position
    0/64).  ACT copies PSUM -> fp16 staging; batched output DMAs
    drain on the otherwise idle GPSIMD SWDGE ring.
  * Host scatters the per-core pair blocks back to V order.

Segment-sharded output means no cross-core reduction; each core
streams ~1/8 of the rows once (~53 MB) and writes 3.2 MB.
"""
import dataclasses

import numpy as np

import concourse.bass as bass
import concourse.mybir as mybir
import concourse.tile as tile
from concourse import bacc
from concourse import bass_utils

P = 128
D = 128
PSEG = 64
OUT_BATCH = 8             # PSUM tiles per output staging DMA
GCH = 48                  # target chunks per gather DMA (~1.6 MB)
N_CORES = 8
F32 = mybir.dt.float32
F16 = mybir.dt.float16
F8 = mybir.dt.float8e4

_CACHE = {}


def _plan_schedule(X, n_cores):
    N = X.shape[0]
    V = int(X.max()) + 1 if N else 1
    perm = np.argsort(X)
    Xs = X[perm].astype(np.int64)

    nws = -(-V // PSEG)
    win_of_node = Xs // PSEG
    counts = np.bincount(win_of_node, minlength=nws)[:nws]
    starts = np.zeros(nws + 1, dtype=np.int64)
    np.cumsum(counts, out=starts[1:])

    # antithetic pairing: largest with smallest -> near-uniform pair sums
    srt = np.argsort(-counts, kind="stable")
    npairs_raw = -(-nws // 2)
    PW = -(-npairs_raw // n_cores)
    npairs = PW * n_cores
    pairA = np.full(npairs, -1, dtype=np.int64)
    pairB = np.full(npairs, -1, dtype=np.int64)
    pairA[:npairs_raw] = srt[:npairs_raw]
    nbw = nws - npairs_raw
    if nbw > 0:
        pairB[:nbw] = srt[::-1][:nbw]
    pcnt = np.where(pairA >= 0, counts[np.clip(pairA, 0, None)], 0) + \
        np.where(pairB >= 0, counts[np.clip(pairB, 0, None)], 0)

    psrt = np.argsort(-pcnt, kind="stable")
    assign = psrt.reshape(PW, n_cores)   # pair index per (slot, core)

    # per (slot, core): cntA, cnt_pair
    aw = pairA[assign]
    bw = pairB[assign]
    cA = np.where(aw >= 0, counts[np.clip(aw, 0, None)], 0)
    cB = np.where(bw >= 0, counts[np.clip(bw, 0, None)], 0)
    cP = cA + cB
    KA = (cA.min(axis=1) // P).astype(np.int64)             # pure-A chunks
    KB = (-(-cA.max(axis=1) // P)).astype(np.int64)         # A ends by here
    KP = np.maximum.reduce([
        np.ones(PW, dtype=np.int64),
        KB,
        (-(-cP.max(axis=1) // P)).astype(np.int64),
    ])
    KA = np.minimum(KA, KP)
    KB = np.minimum(np.maximum(KB, KA), KP)

    # gather groups of pair-slots, <= cap stream chunks; first two small
    off = np.zeros(PW + 1, dtype=np.int64)
    np.cumsum(KP, out=off[1:])
    groups = []
    s0 = 0
    while s0 < PW:
        cap = 16 if len(groups) < 2 else GCH
        s1 = s0
        while s1 < PW and off[s1 + 1] - off[s0] <= cap:
            s1 += 1
        if s1 == s0:
            s1 = s0 + 1
        groups.append((s0, s1))
        s0 = s1

    # stream layout: per group [pure-A blocks][zone blocks][pure-B blocks]
    TOT = int(off[PW])
    chunk_pos = np.zeros((PW, int(KP.max())), dtype=np.int64)
    ginfo = []
    t = 0
    for (s0, s1) in groups:
        t0 = t
        nA = int(sum(KA[s] for s in range(s0, s1)))
        nz = int(sum(KB[s] - KA[s] for s in range(s0, s1)))
        nB_ = int(sum(KP[s] - KB[s] for s in range(s0, s1)))
        pa, pz, pb = t0, t0 + nA, t0 + nA + nz
        for s in range(s0, s1):
            for j in range(int(KP[s])):
                if j < KA[s]:
                    chunk_pos[s, j] = pa
                    pa += 1
                elif j < KB[s]:
                    chunk_pos[s, j] = pz
                    pz += 1
                else:
                    chunk_pos[s, j] = pb
                    pb += 1
        t = t0 + nA + nz + nB_
        ginfo.append(dict(t0=t0, nch=nA + nz + nB_, lenA=nA + nz,
                          zrel=nA, lenB=nz + nB_, s0=s0, s1=s1))
    assert t == TOT

    order = np.full((n_cores, TOT * P), -1, dtype=np.int64)
    xrel = np.full((n_cores, P, TOT), -1.0, dtype=np.float32)
    for c in range(n_cores):
        ov = order[c].reshape(TOT, P)
        xr = xrel[c]
        for s in range(PW):
            rows = []
            vals = []
            for wi, base_off in ((int(aw[s, c]), 0), (int(bw[s, c]), PSEG)):
                if wi < 0:
                    continue
                a, b = int(starts[wi]), int(starts[wi + 1])
                rows.append(perm[a:b])
                vals.append((Xs[a:b] - wi * PSEG + base_off).astype(
                    np.float32))
            kp = int(KP[s])
            rr = np.full(kp * P, -1, dtype=np.int64)
            vv = np.full(kp * P, -1.0, dtype=np.float32)
            if rows:
                rcat = np.concatenate(rows)
                vcat = np.concatenate(vals)
                rr[:len(rcat)] = rcat
                vv[:len(vcat)] = vcat
            rr = rr.reshape(kp, P)
            vv = vv.reshape(kp, P)
            for j in range(kp):
                pos = int(chunk_pos[s, j])
                ov[pos] = rr[j]
                xr[:, pos] = vv[j]

    iota = np.ascontiguousarray(np.broadcast_to(
        np.arange(2 * PSEG, dtype=np.float16)[None, :], (P, 2 * PSEG)))

    return dict(
        V=V, PW=PW, KA=KA, KB=KB, KP=KP, TOT=TOT, n_cores=n_cores,
        groups=groups, ginfo=ginfo, chunk_pos=chunk_pos,
        aw=aw, bw=bw, order=order,
        xrel=xrel.astype(np.float16), iota=iota,
    )


def _make_in_maps(H, meta):
    n_cores, TOT = meta["n_cores"], meta["TOT"]
    H16 = H.astype(np.float16)
    maps = []
    for c in range(n_cores):
        flat = meta["order"][c]
        sel = np.clip(flat, 0, None)
        hh = H16[sel]
        hh[flat < 0] = 0
        hh = np.ascontiguousarray(hh.reshape(TOT, P, D).transpose(1, 0, 2))
        maps.append({
            "h": hh,
            "xrel": meta["xrel"][c],
            "iota": meta["iota"],
        })
    return maps


def _assemble_output(res_outs, meta):
    n_cores, PW, V = meta["n_cores"], meta["PW"], meta["V"]
    aw, bw = meta["aw"], meta["bw"]
    full = np.zeros((-(-V // PSEG) * PSEG + PSEG, D), dtype=np.float32)
    for c in range(n_cores):
        # out layout: [P, PW*D] fp16; partitions 0-63 = A, 64-127 = B
        oc = res_outs[c].astype(np.float32).reshape(P, PW, D)
        for s in range(PW):
            wa, wb = int(aw[s, c]), int(bw[s, c])
            if wa >= 0:
                full[wa * PSEG:(wa + 1) * PSEG] = oc[:PSEG, s]
            if wb >= 0:
                full[wb * PSEG:(wb + 1) * PSEG] = oc[PSEG:, s]
    return full[:V]


def _ap3(ap, mid, inner):
    # replace the free dims of a 2d AP with [mid, inner] ([step, count])
    part = ap.ap[0]
    new = [part, list(mid), list(inner)]
    return dataclasses.replace(ap, ap=new)


def _build_nc(meta, nbufs=6):
    PW, TOT = meta["PW"], meta["TOT"]
    KA = [int(k) for k in meta["KA"]]
    KB = [int(k) for k in meta["KB"]]
    KP = [int(k) for k in meta["KP"]]
    cpos = meta["chunk_pos"]
    n_cores = meta["n_cores"]
    nc = bacc.Bacc("TRN2", target_bir_lowering=False, debug=False,
                   num_devices=n_cores)
    h = nc.dram_tensor("h", [P, TOT, D], F16, kind="ExternalInput").ap()
    xrel_d = nc.dram_tensor("xrel", [P, TOT], F16, kind="ExternalInput").ap()
    iota_d = nc.dram_tensor("iota", [P, 2 * PSEG], F16,
                            kind="ExternalInput").ap()
    out_d = nc.dram_tensor("out", [P, PW * D], F16,
                           kind="ExternalOutput").ap()

    with tile.TileContext(nc) as tc:
        with (
            tc.tile_pool(name="res", bufs=1) as res,
            tc.tile_pool(name="gat", bufs=nbufs) as gat,
            tc.tile_pool(name="oh", bufs=min(24, 2 * len(meta["groups"]))) as ohp,
            tc.tile_pool(name="ps", bufs=4, space="PSUM") as ps,
            tc.tile_pool(name="osb", bufs=3) as osb,
        ):
            xrel_sb = res.tile([P, TOT], F16)
            iota_sb = res.tile([P, 2 * PSEG], F16)
            nc.gpsimd.dma_start(out=xrel_sb[:], in_=xrel_d[:])
            nc.gpsimd.dma_start(out=iota_sb[:], in_=iota_d[:])

            staging = None
            for gi, g in enumerate(meta["ginfo"]):
                t0, nch = g["t0"], g["nch"]
                gt = gat.tile([P, nch * D], F16, tag="gt")
                h1 = nch // 2
                nc.sync.dma_start(
                    out=gt[:, :h1 * D],
                    in_=h[:, t0:t0 + h1, :].rearrange("p t d -> p (t d)"))
                nc.scalar.dma_start(
                    out=gt[:, h1 * D:],
                    in_=h[:, t0 + h1:t0 + nch, :].rearrange(
                        "p t d -> p (t d)"))
                lenA, zrel, lenB = g["lenA"], g["zrel"], g["lenB"]
                ohA = ohp.tile([P, max(lenA, 1) * PSEG], F8, tag="ohA")
                ohB = ohp.tile([P, max(lenB, 1) * PSEG], F8, tag="ohB")
                if lenA:
                    nc.vector.tensor_tensor(
                        out=ohA[:, :lenA * PSEG],
                        in0=_ap3(iota_sb[:, :PSEG], [0, lenA], [1, PSEG]),
                        in1=_ap3(xrel_sb[:, t0:t0 + lenA],
                                 [1, lenA], [0, PSEG]),
                        op=mybir.AluOpType.is_equal,
                    )
                if lenB:
                    nc.vector.tensor_tensor(
                        out=ohB[:, :lenB * PSEG],
                        in0=_ap3(iota_sb[:, PSEG:], [0, lenB], [1, PSEG]),
                        in1=_ap3(xrel_sb[:, t0 + zrel:t0 + zrel + lenB],
                                 [1, lenB], [0, PSEG]),
                        op=mybir.AluOpType.is_equal,
                    )
                for s in range(g["s0"], g["s1"]):
                    pt = ps.tile([P, D], F32, tag="pt")
                    # fallback chunks keep the slice inside the oh tile that
                    # covers them (empty side -> all-pad or ignored output)
                    if KB[s] > 0:
                        a_side = (range(KB[s]), ohA, t0, 0)
                    else:
                        a_side = ([KP[s] - 1], ohB, t0 + zrel, 0)
                    if KP[s] > KA[s]:
                        b_side = (range(KA[s], KP[s]), ohB, t0 + zrel, PSEG)
                    else:
                        b_side = ([KP[s] - 1], ohA, t0, PSEG)
                    for chunks, oh, rng0, poff in (a_side, b_side):
                        for ji, j in enumerate(chunks):
                            pos = int(cpos[s, j])
                            rel = pos - rng0
                            nc.tensor.matmul(
                                out=pt[poff:poff + PSEG, :],
                                lhsT=oh[:, rel * PSEG:(rel + 1) * PSEG],
                                rhs=gt[:, (pos - t0) * D:(pos - t0 + 1) * D],
                                start=(ji == 0),
                                stop=(ji == len(chunks) - 1),
                                tile_position=(0, poff),
                            )
                    b = s % OUT_BATCH
                    if b == 0:
                        staging = osb.tile([P, OUT_BATCH * D], F16, tag="st")
                    nc.scalar.copy(out=staging[:, b * D:(b + 1) * D],
                                   in_=pt[:])
                    if b == OUT_BATCH - 1 or s == PW - 1:
                        g0 = s - b
                        nc.gpsimd.dma_start(
                            out=out_d[:, g0 * D:(s + 1) * D],
                            in_=staging[:, :(b + 1) * D])

    nc.compile()
    return nc


def prepare(H, X_node):
    """Plan + build + shard. Returns (nc, in_maps, meta). Cached on the
    schedule signature so repeated kernel() calls reuse the compiled
    program."""
    H = np.ascontiguousarray(np.asarray(H, dtype=np.float32))
    X = np.asarray(X_node).astype(np.int64)
    assert H.ndim == 2 and H.shape[1] == D and X.shape == (H.shape[0],)

    meta = _plan_schedule(X, N_CORES)
    key = (meta["PW"], meta["TOT"],
           tuple(int(k) for k in meta["KA"]),
           tuple(int(k) for k in meta["KB"]),
           tuple(int(k) for k in meta["KP"]))
    if key not in _CACHE:
        _CACHE[key] = _build_nc(meta)
    nc = _CACHE[key]
    in_maps = _make_in_maps(H, meta)
    return nc, in_maps, meta


def kernel(H, X_node):
    nc, in_maps, meta = prepare(H, X_node)
    res = bass_utils.run_bass_kernel_spmd(
        nc, in_maps, core_ids=list(range(N_CORES)))
    out = _assemble_output([res.results[c]["out"] for c in range(N_CORES)],
                           meta)
    return out.astype(np.float32)
